# revision 22
# baseline (speedup 1.0000x reference)
"""BuildingGCN Trainium2 kernel: 3-layer GCN + global mean pool + MLP head,
distributed over 8 NeuronCores.

v2 strategy per core (node shard = 12500 nodes, edges sharded by dst owner,
self-loops materialized as explicit edges):
  y tables [100352, 128] bf16 in DRAM (AllGathered each layer; 256B rows =
  dma_gather minimum elem). Windows of 128 dst nodes are processed in groups
  of 8; per (group, chunk) one dma_gather call fills a bf16 gt tile, a bf16
  band matrix is generated on DVE (is_equal vs iota), and segment-sum runs as
  band-matmul accumulation on TensorE (bf16 -> FWL fast weight load) into
  PSUM banks holding 4 windows each. Post-ops: z = dinv*S + b (DVE affine),
  h = relu (ACT, bf16 out), transpose (PE), xw = hT @ W (bf16), y = dinv*xw
  (DVE, bf16) -> next table. Pool via one-hot matmul accumulated in PSUM.
"""
import sys
import types

from dataclasses import dataclass, field

import numpy as np

import concourse.bass as bass
import concourse.tile as tile
from concourse import bacc, mybir
from concourse._compat import cdiv
from concourse.bass_utils import run_bass_kernel_spmd

P = 128
F32 = mybir.dt.float32
F16 = mybir.dt.float16
I16 = mybir.dt.int16
NPF16 = np.float16


@dataclass
class Cfg:
    n_nodes: int = 100000
    n_cores: int = 8
    n_graphs: int = 256
    gw: int = 3                 # windows per group
    gslots: int = 64            # graph slots per core
    in_ch: int = 8
    c1: int = 64
    c2: int = 128
    c3: int = 64
    ch1: int = 32               # MLP hidden
    tw: int = 128               # table row width (bf16 elems; 256B)

    @property
    def ns(self):
        return self.n_nodes // self.n_cores          # 12500

    @property
    def nw(self):
        return cdiv(self.ns, P)                      # 98

    @property
    def nsp(self):
        return self.nw * P                           # 12544 padded shard rows

    @property
    def chunk(self):
        return self.nsp * self.n_cores // 4          # 25088 table rows

    @property
    def ng(self):
        return cdiv(self.nw, self.gw)                # 13 groups

    @property
    def w_last(self):
        return self.ns - (self.nw - 1) * P


@dataclass
class Plan:
    cfg: Cfg
    nbwk: np.ndarray          # [NW, 4] blocks per (window, chunk), cross-core max
    g0: list = field(default_factory=list)     # per-core first graph id
    gwid: list = field(default_factory=list)   # per-core graph width
    # derived (all indexed [g][k])
    nb_gk: list = field(default_factory=list)      # blocks per call
    icol0: list = field(default_factory=list)      # idx_all col offset of call
    b0_gk: list = field(default_factory=list)      # seg_all block offset of call
    gcol0: list = field(default_factory=list)      # idx col offset of group
    gcols: list = field(default_factory=list)      # idx cols of group
    # [g][k][wl] -> (block_off_in_call, nblocks)
    wblk: list = field(default_factory=list)

    def finalize(self):
        cfg = self.cfg
        icol = 0
        b0 = 0
        for g in range(cfg.ng):
            ws = list(range(g * cfg.gw, min((g + 1) * cfg.gw, cfg.nw)))
            self.gcol0.append(icol)
            nbs, cols, boffs, wb = [], [], [], []
            for k in range(4):
                off = 0
                wbk = []
                for w in ws:
                    nb = int(self.nbwk[w, k])
                    wbk.append((off, nb))
                    off += nb
                nbs.append(off)
                cols.append(icol)
                boffs.append(b0)
                wb.append(wbk)
                icol += off * P // 16
                b0 += off
            self.nb_gk.append(nbs)
            self.icol0.append(cols)
            self.b0_gk.append(boffs)
            self.wblk.append(wb)
            self.gcols.append(icol - self.gcol0[-1])
        self.idx_cols = icol
        self.nb_total = b0


def preprocess(cfg: Cfg, edge_index, batch):
    """Host-side graph partitioning. Returns (plan, shared, per-core dicts)."""
    src = np.asarray(edge_index[0], dtype=np.int64)
    dst = np.asarray(edge_index[1], dtype=np.int64)
    batch = np.asarray(batch, dtype=np.int64)
    N, NC, NS, NW = cfg.n_nodes, cfg.n_cores, cfg.ns, cfg.nw

    deg = (np.bincount(dst, minlength=N) + 1.0).astype(np.float32)
    dinv = (1.0 / np.sqrt(deg)).astype(np.float32)
    cnt = np.bincount(batch, minlength=cfg.n_graphs).astype(np.float32)
    invcnt = (1.0 / np.maximum(cnt, 1.0)).astype(np.float32)

    # self-loops as explicit edges
    loops = np.arange(N, dtype=np.int64)
    src = np.concatenate([src, loops])
    dst = np.concatenate([dst, loops])

    # global table row of a source node (shards padded to NSP rows)
    g_row = (src // NS) * cfg.nsp + (src % NS)

    per_core_edges = []
    counts = np.zeros((NC, NW, 4), dtype=np.int64)
    for c in range(NC):
        m = (dst >= c * NS) & (dst < (c + 1) * NS)
        s_r = g_row[m]
        d = dst[m] - c * NS
        w = d // P
        seg = (d % P).astype(np.float32)
        k = s_r // cfg.chunk
        sl = (s_r % cfg.chunk).astype(np.int16)
        g = w // cfg.gw
        key = (g * 4 + k) * NW + w
        order = np.argsort(key, kind="stable")
        sl, seg, w, k = sl[order], seg[order], w[order], k[order]
        cnts = np.bincount((k * NW + w), minlength=4 * NW)
        counts[c] = cnts.reshape(4, NW).T
        per_core_edges.append((sl, seg))

    nbwk = np.ceil(counts.max(axis=0) / P).astype(np.int64)  # [NW, 4]
    plan = Plan(cfg, nbwk)
    plan.g0 = [int(batch[c * NS]) for c in range(NC)]
    plan.gwid = [int(batch[(c + 1) * NS - 1]) - plan.g0[c] + 1
                 for c in range(NC)]
    assert max(plan.gwid) <= cfg.gslots
    plan.finalize()

    shared = {
        "iota_t": np.tile(np.arange(P, dtype=np.float32),
                          (P, 1)).astype(NPF16),
        "ident_t": np.eye(P, dtype=np.float32).astype(NPF16),
        "invcnt_r": np.tile(invcnt, (cfg.c3, 1)).astype(np.float32),
    }

    cores = []
    for c in range(NC):
        sl, seg = per_core_edges[c]
        # per-core cell start offsets in the sorted stream
        cell_cnt = counts[c]                     # [NW, 4]
        idx_stream = np.zeros(plan.nb_total * P, dtype=np.int16)
        seg_stream = np.full(plan.nb_total * P, -1.0, dtype=np.float32)
        pos = 0
        for g in range(cfg.ng):
            ws = list(range(g * cfg.gw, min((g + 1) * cfg.gw, cfg.nw)))
            for k in range(4):
                spos = plan.b0_gk[g][k] * P
                for w in ws:
                    n = int(cell_cnt[w, k])
                    nslot = int(nbwk[w, k]) * P
                    idx_stream[spos:spos + n] = sl[pos:pos + n]
                    seg_stream[spos:spos + n] = seg[pos:pos + n]
                    pos += n
                    spos += nslot
        assert pos == len(sl)
        idx_all = np.tile(idx_stream.reshape(-1, 16).T, (8, 1))  # [128, cols]
        seg_all = seg_stream.reshape(-1, P).T.copy().astype(NPF16)

        nodes = np.arange(c * NS, (c + 1) * NS)
        dinv_t = np.ones((P, NW), dtype=np.float32)
        gslot_t = np.full((P, NW), -1.0, dtype=np.float32)
        fl_d = dinv_t.T.reshape(-1)
        fl_d[:NS] = dinv[nodes]
        fl_g = gslot_t.T.reshape(-1)
        fl_g[:NS] = (batch[nodes] - plan.g0[c]).astype(np.float32)
        cores.append({
            "idx_all": idx_all,
            "seg_all": seg_all,
            "dinv_t": fl_d.reshape(NW, P).T.copy(),
            "gslot_t": fl_g.reshape(NW, P).T.astype(NPF16).copy(),
        })
    return plan, shared, cores


def build_program(plan: Plan, n_cores: int):
    cfg = plan.cfg
    NW, NG, GW = cfg.nw, cfg.ng, cfg.gw
    nc = bacc.Bacc("TRN2", target_bir_lowering=False, debug=False,
                   num_devices=n_cores, num_swdge_queues=4,
                   dynamic_dma_scratch_size=32768)

    def din(name, shape, dt=F32):
        return nc.dram_tensor(name, shape, dt, kind="ExternalInput").ap()

    xT = din("xT", [cfg.in_ch, cfg.nsp])
    w1 = din("w1", [cfg.in_ch, cfg.c1])
    w2 = din("w2b", [cfg.c1, cfg.c2], F16)
    w3 = din("w3b", [cfg.c2, cfg.c3], F16)
    wl1 = din("wl1", [cfg.c3, cfg.ch1])
    wl2 = din("wl2", [cfg.ch1, 1])
    b1r = din("b1r", [P, cfg.c1])
    b2r = din("b2r", [P, cfg.c2])
    b3r = din("b3r", [P, cfg.c3])
    bl1c = din("bl1c", [cfg.ch1, 1])
    bl2c = din("bl2c", [1, 1])
    dinv_d = din("dinv_t", [P, NW])
    gslot_d = din("gslot_t", [P, NW], F16)
    invcnt_d = din("invcnt_r", [cfg.c3, cfg.n_graphs])
    iota_d = din("iota_t", [P, P], F16)
    ident_d = din("ident_t", [P, P], F16)
    idx_d = din("idx_all", [P, plan.idx_cols], I16)
    seg_d = din("seg_all", [P, plan.nb_total], F16)
    out_d = nc.dram_tensor("out", [1, cfg.n_graphs], F32,
                           kind="ExternalOutput").ap()

    rg = [list(range(n_cores))]

    with tile.TileContext(nc) as tc:
        with tc.tile_pool(name="const", bufs=1) as cp, \
             tc.tile_pool(name="gath", bufs=2) as gp, \
             tc.tile_pool(name="band", bufs=2) as bp, \
             tc.tile_pool(name="idx", bufs=8) as ip, \
             tc.tile_pool(name="win", bufs=4) as wp, \
             tc.tile_pool(name="psS", bufs=2, space="PSUM") as ps_s, \
             tc.tile_pool(name="psT", bufs=2, space="PSUM") as ps_t, \
             tc.tile_pool(name="psX", bufs=1, space="PSUM") as ps_x, \
             tc.tile_pool(name="psP", bufs=1, space="PSUM") as ps_p, \
             tc.tile_pool(name="dram", bufs=1, space="DRAM") as dp:

            def load_const(name, ap, shape, dt=F32):
                t = cp.tile(shape, dt, tag=name)
                nc.sync.dma_start(t[:], ap[:])
                return t

            w1_s = load_const("w1", w1, [cfg.in_ch, cfg.c1])
            w2_s = load_const("w2", w2, [cfg.c1, cfg.c2], F16)
            w3_s = load_const("w3", w3, [cfg.c2, cfg.c3], F16)
            wl1_s = load_const("wl1", wl1, [cfg.c3, cfg.ch1])
            wl2_s = load_const("wl2", wl2, [cfg.ch1, 1])
            b1_s = load_const("b1", b1r, [P, cfg.c1])
            b2_s = load_const("b2", b2r, [P, cfg.c2])
            b3_s = load_const("b3", b3r, [P, cfg.c3])
            bl1_s = load_const("bl1", bl1c, [cfg.ch1, 1])
            bl2_s = load_const("bl2", bl2c, [1, 1])
            dinv_s = load_const("dinv", dinv_d, [P, NW])
            gslot_s = load_const("gslot", gslot_d, [P, NW], F16)
            invcnt_s = load_const("invcnt", invcnt_d, [cfg.c3, cfg.n_graphs])
            iota_s = load_const("iota", iota_d, [P, P], F16)
            ident_s = load_const("ident", ident_d, [P, P], F16)
            seg_s = load_const("seg", seg_d, [P, plan.nb_total], F16)

            y_shA = dp.tile([cfg.nsp, cfg.tw], F16)
            y_shB = dp.tile([cfg.nsp, cfg.tw], F16)
            yfullA = dp.tile([cfg.nsp * n_cores, cfg.tw], F16)
            yfullB = dp.tile([cfg.nsp * n_cores, cfg.tw], F16)
            yfullC = dp.tile([cfg.nsp * n_cores, cfg.tw], F16)
            pool_sh = dp.tile([cfg.c3, cfg.gslots], F32)
            pool_ag = dp.tile([cfg.c3 * n_cores, cfg.gslots], F32)

            def ts(out, in0, s1, op0, s2=None, op1=mybir.AluOpType.bypass):
                nc.vector.tensor_scalar(out, in0, s1, s2, op0, op1)

            # ---- phase 0: y1 = dinv * (x @ W1) -> table A ----
            for w in range(NW):
                xt_w = wp.tile([cfg.in_ch, P], F32, tag="xtw")
                nc.sync.dma_start(xt_w[:], xT[:, w * P:(w + 1) * P])
                ps = ps_x.tile([P, cfg.c1], F32, tag="xw")
                nc.tensor.matmul(ps[:], lhsT=xt_w[:],
                                 rhs=w1_s[:], start=True, stop=True)
                y_t = wp.tile([P, cfg.c1], F16, tag="yt")
                ts(y_t[:], ps[:], dinv_s[:, w:w + 1], mybir.AluOpType.mult)
                nc.sync.dma_start(y_shA[w * P:(w + 1) * P, :cfg.c1], y_t[:])

            nc.gpsimd.collective_compute(
                "AllGather", mybir.AluOpType.bypass, replica_groups=rg,
                ins=[y_shA.opt()], outs=[yfullA.opt()])

            # ---- aggregation layers ----
            # full PSUM bank: start=True clears has_written for the WHOLE
            # bank, so each logical accumulator owns a bank exclusively and
            # gets exactly one start (first matmul) / stop (last matmul).
            pool_pp = ps_p.tile([P, 512], F32, tag="pool")

            def agg_layer(lnum, cin, cout, ytab, b_s, wnext_s, ysh_next):
                for g in range(NG):
                    ws = list(range(g * GW, min((g + 1) * GW, NW)))
                    Sa = ps_s.tile([P, 512], F32, tag="Sa",
                                   name=f"Sa_{lnum}_{g}")
                    Sb = None
                    if len(ws) * cin > 512:
                        Sb = ps_s.tile([P, 512], F32, tag="Sb",
                                       name=f"Sb_{lnum}_{g}")
                    wpb = 512 // cin           # windows per bank
                    # call k's blocks live at group-tile offset koff[k]
                    koff = [plan.b0_gk[g][k] - plan.b0_gk[g][0]
                            for k in range(4)]
                    nbg = koff[3] + plan.nb_gk[g][3]

                    # first/last matmul (k, j) PER BANK for start/stop flags
                    first, last = {}, {}
                    for k in range(4):
                        for wl, w in enumerate(ws):
                            off, nb = plan.wblk[g][k][wl]
                            if nb == 0:
                                continue
                            bk = wl // wpb
                            if bk not in first:
                                first[bk] = (k, wl, off)
                            last[bk] = (k, wl, off + nb - 1)

                    gt = gp.tile([P, nbg, cfg.tw], F16, tag="gt",
                                 name=f"gt_{lnum}_{g}")
                    for k in range(4):
                        nb = plan.nb_gk[g][k]
                        if nb == 0:
                            continue
                        ni = nb * P
                        cols = ni // 16
                        it = ip.tile([P, cols], I16, tag="idxt")
                        ic0 = plan.icol0[g][k]
                        nc.sync.dma_start(it[:], idx_d[:, ic0:ic0 + cols])
                        nc.gpsimd.dma_gather(
                            gt[:, koff[k]:koff[k] + nb, :],
                            ytab[k * cfg.chunk:(k + 1) * cfg.chunk, :],
                            it[:], ni, ni, cfg.tw,
                            single_packet=False, queue_num=k)
                    band = bp.tile([P, nbg, P], F16, tag="band",
                                   name=f"band_{lnum}_{g}")
                    sg0 = plan.b0_gk[g][0]
                    nc.vector.tensor_tensor(
                        out=band[:],
                        in0=seg_s[:, sg0:sg0 + nbg].unsqueeze(2)
                            .broadcast_to([P, nbg, P]),
                        in1=iota_s[:].unsqueeze(1)
                            .broadcast_to([P, nbg, P]),
                        op=mybir.AluOpType.is_equal)
                    for k in range(4):
                        for wl, w in enumerate(ws):
                            off, nbw = plan.wblk[g][k][wl]
                            bk = wl // wpb
                            S = Sa if bk == 0 else Sb
                            col = (wl % wpb) * cin
                            for j in range(off, off + nbw):
                                nc.tensor.matmul(
                                    S[:, col:col + cin],
                                    lhsT=band[:, koff[k] + j, :],
                                    rhs=gt[:, koff[k] + j, :cin],
                                    start=(first[bk] == (k, wl, j)),
                                    stop=(last[bk] == (k, wl, j)),
                                    skip_group_check=True)

                    for wl, w in enumerate(ws):
                        bk = wl // wpb
                        S = Sa if bk == 0 else Sb
                        col = (wl % wpb) * cin
                        z = wp.tile([P, cin], F32, tag="z")
                        nc.vector.affine_then_add(
                            z[:], S[:, col:col + cin], b_s[:],
                            dinv_s[:, w:w + 1], 0.0)
                        h = wp.tile([P, cin], F16, tag="h")
                        nc.scalar.activation(
                            h[:], z[:], mybir.ActivationFunctionType.Relu)
                        if lnum < 3:
                            pst = ps_t.tile([cin, P], F16, tag="tp")
                            nc.tensor.transpose(pst[:], h[:], ident_s[:])
                            hT = wp.tile([cin, P], F16, tag="hT")
                            nc.vector.tensor_copy(hT[:], pst[:])
                            ps = ps_x.tile([P, cout], F32, tag="xw")
                            nc.tensor.matmul(ps[:], lhsT=hT[:], rhs=wnext_s[:],
                                             start=True, stop=True)
                            y_t = wp.tile([P, cout], F16, tag="yt")
                            ts(y_t[:], ps[:], dinv_s[:, w:w + 1],
                               mybir.AluOpType.mult)
                            nc.sync.dma_start(
                                ysh_next[w * P:(w + 1) * P, :cout], y_t[:])
                        else:
                            gb = wp.tile([P, cfg.gslots], F16, tag="gb")
                            nc.vector.tensor_tensor(
                                out=gb[:],
                                in0=gslot_s[:, w:w + 1]
                                    .broadcast_to([P, cfg.gslots]),
                                in1=iota_s[:, :cfg.gslots],
                                op=mybir.AluOpType.is_equal)
                            nc.tensor.matmul(
                                pool_pp[:cfg.c3, :cfg.gslots], lhsT=h[:],
                                rhs=gb[:], start=(w == 0), stop=(w == NW - 1),
                                skip_group_check=True)

            agg_layer(1, cfg.c1, cfg.c2, yfullA, b1_s, w2_s, y_shB)
            nc.gpsimd.collective_compute(
                "AllGather", mybir.AluOpType.bypass, replica_groups=rg,
                ins=[y_shB.opt()], outs=[yfullB.opt()])
            agg_layer(2, cfg.c2, cfg.c3, yfullB, b2_s, w3_s, y_shA)
            nc.gpsimd.collective_compute(
                "AllGather", mybir.AluOpType.bypass, replica_groups=rg,
                ins=[y_shA.opt()], outs=[yfullC.opt()])
            agg_layer(3, cfg.c3, None, yfullC, b3_s, None, None)

            # ---- pooling finale ----
            pool_t = cp.tile([cfg.c3, cfg.gslots], F32)
            nc.vector.tensor_copy(pool_t[:], pool_pp[:cfg.c3, :cfg.gslots])
            nc.sync.dma_start(pool_sh[:], pool_t[:])
            nc.gpsimd.collective_compute(
                "AllGather", mybir.AluOpType.bypass, replica_groups=rg,
                ins=[pool_sh.opt()], outs=[pool_ag.opt()])
            M = cp.tile([cfg.c3, cfg.n_graphs], F32)
            nc.vector.memset(M[:], 0.0)
            for c in range(n_cores):
                agc = wp.tile([cfg.c3, cfg.gslots], F32, tag="agc")
                nc.sync.dma_start(agc[:],
                                  pool_ag[c * cfg.c3:(c + 1) * cfg.c3, :])
                g0 = plan.g0[c]
                wdt = min(plan.gwid[c], cfg.n_graphs - g0)
                nc.vector.tensor_tensor(M[:, g0:g0 + wdt], M[:, g0:g0 + wdt],
                                        agc[:, :wdt], mybir.AluOpType.add)
            M2 = cp.tile([cfg.c3, cfg.n_graphs], F32)
            nc.vector.tensor_tensor(M2[:], M[:], invcnt_s[:],
                                    mybir.AluOpType.mult)
            ps1 = ps_t.tile([cfg.ch1, cfg.n_graphs], F32, tag="tp")
            nc.tensor.matmul(ps1[:], lhsT=wl1_s[:], rhs=M2[:],
                             start=True, stop=True)
            g1 = cp.tile([cfg.ch1, cfg.n_graphs], F32)
            ts(g1[:], ps1[:], bl1_s[:, 0:1], mybir.AluOpType.add, 0.0,
               mybir.AluOpType.max)
            ps2 = ps_x.tile([1, cfg.n_graphs], F32, tag="xw")
            nc.tensor.matmul(ps2[:], lhsT=wl2_s[:], rhs=g1[:],
                             start=True, stop=True)
            osb = cp.tile([1, cfg.n_graphs], F32)
            ts(osb[:], ps2[:], bl2_s[:, 0:1], mybir.AluOpType.add)
            nc.sync.dma_start(out_d[:], osb[:])

    nc.compile()
    return nc


def make_in_maps(cfg, plan, shared, cores, x, W1, b1, W2, b2, W3, b3,
                 Wl1, bl1, Wl2, bl2):
    NS = cfg.ns
    x = np.asarray(x, dtype=np.float32)
    com = {
        "w1": np.asarray(W1, np.float32),
        "w2b": np.asarray(W2, np.float32).astype(NPF16),
        "w3b": np.asarray(W3, np.float32).astype(NPF16),
        "wl1": np.asarray(Wl1, np.float32), "wl2": np.asarray(Wl2, np.float32),
        "b1r": np.tile(np.asarray(b1, np.float32), (P, 1)),
        "b2r": np.tile(np.asarray(b2, np.float32), (P, 1)),
        "b3r": np.tile(np.asarray(b3, np.float32), (P, 1)),
        "bl1c": np.asarray(bl1, np.float32).reshape(-1, 1),
        "bl2c": np.asarray(bl2, np.float32).reshape(1, 1),
        "invcnt_r": shared["invcnt_r"], "iota_t": shared["iota_t"],
        "ident_t": shared["ident_t"],
    }
    in_maps = []
    for c in range(cfg.n_cores):
        m = dict(com)
        xs = np.zeros((cfg.in_ch, cfg.nsp), dtype=np.float32)
        xs[:, :NS] = x[c * NS:(c + 1) * NS].T
        m["xT"] = xs
        m.update(cores[c])
        in_maps.append(m)
    return in_maps


_CACHE = {}


def _install_profile_hook():
    try:
        import antenv.axon_hooks  # noqa: F401
        return
    except ImportError:
        pass
    try:
        mod = types.ModuleType("antenv.axon_hooks")
        _h = [None]
        mod.set_axon_ntff_profile_hook = lambda h: _h.__setitem__(0, h)
        mod.get_axon_ntff_profile_hook = lambda: _h[0]
        sys.modules["antenv.axon_hooks"] = mod
        from trn_agent_boot.trn_boot import _ntff_profile_via_ctypes
        mod.set_axon_ntff_profile_hook(
            _ntff_profile_via_ctypes("/opt/axon/libaxon_pjrt.so"))
    except Exception:
        pass


def run(cfg, x, edge_index, batch, W1, b1, W2, b2, W3, b3, Wl1, bl1, Wl2, bl2,
        trace=False):
    plan, shared, cores = preprocess(cfg, edge_index, batch)
    key = ("prog", cfg.n_nodes, plan.nb_total, plan.idx_cols,
           tuple(plan.g0), tuple(plan.gwid))
    if key not in _CACHE:
        _CACHE[key] = build_program(plan, cfg.n_cores)
    nc = _CACHE[key]
    in_maps = make_in_maps(cfg, plan, shared, cores, x, W1, b1, W2, b2,
                           W3, b3, Wl1, bl1, Wl2, bl2)
    if trace:
        _install_profile_hook()
    res = run_bass_kernel_spmd(nc, in_maps, list(range(cfg.n_cores)),
                               trace=trace)
    out = np.asarray(res.results[0]["out"]).reshape(-1)[:cfg.n_graphs]
    return out.astype(np.float32), res


def kernel(x, edge_index, batch, W1, b1, W2, b2, W3, b3, Wl1, bl1, Wl2, bl2):
    cfg = Cfg()
    out, _ = run(cfg, x, edge_index, batch, W1, b1, W2, b2, W3, b3,
                 Wl1, bl1, Wl2, bl2)
    return out


# revision 24
# speedup vs baseline: 1.2111x; 1.2111x over previous
"""BuildingGCN Trainium2 kernel: 3-layer GCN + global mean pool + MLP head,
distributed over 8 NeuronCores.

v2 strategy per core (node shard = 12500 nodes, edges sharded by dst owner,
self-loops materialized as explicit edges):
  y tables [100352, 128] bf16 in DRAM (AllGathered each layer; 256B rows =
  dma_gather minimum elem). Windows of 128 dst nodes are processed in groups
  of 8; per (group, chunk) one dma_gather call fills a bf16 gt tile, a bf16
  band matrix is generated on DVE (is_equal vs iota), and segment-sum runs as
  band-matmul accumulation on TensorE (bf16 -> FWL fast weight load) into
  PSUM banks holding 4 windows each. Post-ops: z = dinv*S + b (DVE affine),
  h = relu (ACT, bf16 out), transpose (PE), xw = hT @ W (bf16), y = dinv*xw
  (DVE, bf16) -> next table. Pool via one-hot matmul accumulated in PSUM.
"""
import sys
import types

from dataclasses import dataclass, field

import numpy as np

import concourse.bass as bass
import concourse.tile as tile
from concourse import bacc, mybir
from concourse._compat import cdiv
from concourse.bass_utils import run_bass_kernel_spmd

P = 128
F32 = mybir.dt.float32
F16 = mybir.dt.float16
I16 = mybir.dt.int16
NPF16 = np.float16


@dataclass
class Cfg:
    n_nodes: int = 100000
    n_cores: int = 8
    n_graphs: int = 256
    gw: int = 2                 # windows per group
    gslots: int = 64            # graph slots per core
    in_ch: int = 8
    c1: int = 64
    c2: int = 128
    c3: int = 64
    ch1: int = 32               # MLP hidden
    tw: int = 128               # table row width (bf16 elems; 256B)

    @property
    def ns(self):
        return self.n_nodes // self.n_cores          # 12500

    @property
    def nw(self):
        return cdiv(self.ns, P)                      # 98

    @property
    def nsp(self):
        return self.nw * P                           # 12544 padded shard rows

    @property
    def chunk(self):
        return self.nsp * self.n_cores // 4          # 25088 table rows

    @property
    def ng(self):
        return cdiv(self.nw, self.gw)                # 13 groups

    @property
    def w_last(self):
        return self.ns - (self.nw - 1) * P


@dataclass
class Plan:
    cfg: Cfg
    nbwk: np.ndarray          # [NW, 4] blocks per (window, chunk), cross-core max
    g0: list = field(default_factory=list)     # per-core first graph id
    gwid: list = field(default_factory=list)   # per-core graph width
    # derived (all indexed [g][k])
    nb_gk: list = field(default_factory=list)      # blocks per call
    icol0: list = field(default_factory=list)      # idx_all col offset of call
    b0_gk: list = field(default_factory=list)      # seg_all block offset of call
    gcol0: list = field(default_factory=list)      # idx col offset of group
    gcols: list = field(default_factory=list)      # idx cols of group
    # [g][k][wl] -> (block_off_in_call, nblocks)
    wblk: list = field(default_factory=list)

    def finalize(self):
        cfg = self.cfg
        icol = 0
        b0 = 0
        for g in range(cfg.ng):
            ws = list(range(g * cfg.gw, min((g + 1) * cfg.gw, cfg.nw)))
            self.gcol0.append(icol)
            nbs, cols, boffs, wb = [], [], [], []
            for k in range(4):
                off = 0
                wbk = []
                for w in ws:
                    nb = int(self.nbwk[w, k])
                    wbk.append((off, nb))
                    off += nb
                nbs.append(off)
                cols.append(icol)
                boffs.append(b0)
                wb.append(wbk)
                icol += off * P // 16
                b0 += off
            self.nb_gk.append(nbs)
            self.icol0.append(cols)
            self.b0_gk.append(boffs)
            self.wblk.append(wb)
            self.gcols.append(icol - self.gcol0[-1])
        self.idx_cols = icol
        self.nb_total = b0


def preprocess(cfg: Cfg, edge_index, batch):
    """Host-side graph partitioning. Returns (plan, shared, per-core dicts)."""
    src = np.asarray(edge_index[0], dtype=np.int64)
    dst = np.asarray(edge_index[1], dtype=np.int64)
    batch = np.asarray(batch, dtype=np.int64)
    N, NC, NS, NW = cfg.n_nodes, cfg.n_cores, cfg.ns, cfg.nw

    deg = (np.bincount(dst, minlength=N) + 1.0).astype(np.float32)
    dinv = (1.0 / np.sqrt(deg)).astype(np.float32)
    cnt = np.bincount(batch, minlength=cfg.n_graphs).astype(np.float32)
    invcnt = (1.0 / np.maximum(cnt, 1.0)).astype(np.float32)

    # self-loops as explicit edges
    loops = np.arange(N, dtype=np.int64)
    src = np.concatenate([src, loops])
    dst = np.concatenate([dst, loops])

    # global table row of a source node. Tables are split: top half-shards
    # (local rows < HALF) of all cores at rows [0, NC*HALF), bottom
    # half-shards after, so each half can be AllGathered independently.
    HALF = cfg.nsp // 2
    own = src // NS
    r = src % NS
    g_row = np.where(r < HALF, own * HALF + r,
                     NC * HALF + own * HALF + (r - HALF))

    per_core_edges = []
    counts = np.zeros((NC, NW, 4), dtype=np.int64)
    for c in range(NC):
        m = (dst >= c * NS) & (dst < (c + 1) * NS)
        s_r = g_row[m]
        d = dst[m] - c * NS
        w = d // P
        seg = (d % P).astype(np.float32)
        k = s_r // cfg.chunk
        sl = (s_r % cfg.chunk).astype(np.int16)
        g = w // cfg.gw
        key = (g * 4 + k) * NW + w
        order = np.argsort(key, kind="stable")
        sl, seg, w, k = sl[order], seg[order], w[order], k[order]
        cnts = np.bincount((k * NW + w), minlength=4 * NW)
        counts[c] = cnts.reshape(4, NW).T
        per_core_edges.append((sl, seg))

    nbwk = np.ceil(counts.max(axis=0) / P).astype(np.int64)  # [NW, 4]
    plan = Plan(cfg, nbwk)
    plan.g0 = [int(batch[c * NS]) for c in range(NC)]
    plan.gwid = [int(batch[(c + 1) * NS - 1]) - plan.g0[c] + 1
                 for c in range(NC)]
    assert max(plan.gwid) <= cfg.gslots
    plan.finalize()

    shared = {
        "iota_t": np.tile(np.arange(P, dtype=np.float32),
                          (P, 1)).astype(NPF16),
        "ident_t": np.eye(P, dtype=np.float32).astype(NPF16),
        "invcnt_r": np.tile(invcnt, (cfg.c3, 1)).astype(np.float32),
    }

    cores = []
    for c in range(NC):
        sl, seg = per_core_edges[c]
        # per-core cell start offsets in the sorted stream
        cell_cnt = counts[c]                     # [NW, 4]
        idx_stream = np.zeros(plan.nb_total * P, dtype=np.int16)
        seg_stream = np.full(plan.nb_total * P, -1.0, dtype=np.float32)
        pos = 0
        for g in range(cfg.ng):
            ws = list(range(g * cfg.gw, min((g + 1) * cfg.gw, cfg.nw)))
            for k in range(4):
                spos = plan.b0_gk[g][k] * P
                for w in ws:
                    n = int(cell_cnt[w, k])
                    nslot = int(nbwk[w, k]) * P
                    idx_stream[spos:spos + n] = sl[pos:pos + n]
                    seg_stream[spos:spos + n] = seg[pos:pos + n]
                    pos += n
                    spos += nslot
        assert pos == len(sl)
        idx_all = np.tile(idx_stream.reshape(-1, 16).T, (8, 1))  # [128, cols]
        seg_all = seg_stream.reshape(-1, P).T.copy().astype(NPF16)

        nodes = np.arange(c * NS, (c + 1) * NS)
        dinv_t = np.ones((P, NW), dtype=np.float32)
        gslot_t = np.full((P, NW), -1.0, dtype=np.float32)
        fl_d = dinv_t.T.reshape(-1)
        fl_d[:NS] = dinv[nodes]
        fl_g = gslot_t.T.reshape(-1)
        fl_g[:NS] = (batch[nodes] - plan.g0[c]).astype(np.float32)
        cores.append({
            "idx_all": idx_all,
            "seg_all": seg_all,
            "dinv_t": fl_d.reshape(NW, P).T.copy(),
            "gslot_t": fl_g.reshape(NW, P).T.astype(NPF16).copy(),
        })
    return plan, shared, cores


def build_program(plan: Plan, n_cores: int):
    cfg = plan.cfg
    NW, NG, GW = cfg.nw, cfg.ng, cfg.gw
    nc = bacc.Bacc("TRN2", target_bir_lowering=False, debug=False,
                   num_devices=n_cores, num_swdge_queues=4,
                   dynamic_dma_scratch_size=32768)

    def din(name, shape, dt=F32):
        return nc.dram_tensor(name, shape, dt, kind="ExternalInput").ap()

    xT = din("xT", [cfg.in_ch, cfg.nsp])
    w1 = din("w1", [cfg.in_ch, cfg.c1])
    w2 = din("w2b", [cfg.c1, cfg.c2], F16)
    w3 = din("w3b", [cfg.c2, cfg.c3], F16)
    wl1 = din("wl1", [cfg.c3, cfg.ch1])
    wl2 = din("wl2", [cfg.ch1, 1])
    b1r = din("b1r", [P, cfg.c1])
    b2r = din("b2r", [P, cfg.c2])
    b3r = din("b3r", [P, cfg.c3])
    bl1c = din("bl1c", [cfg.ch1, 1])
    bl2c = din("bl2c", [1, 1])
    dinv_d = din("dinv_t", [P, NW])
    gslot_d = din("gslot_t", [P, NW], F16)
    invcnt_d = din("invcnt_r", [cfg.c3, cfg.n_graphs])
    iota_d = din("iota_t", [P, P], F16)
    ident_d = din("ident_t", [P, P], F16)
    idx_d = din("idx_all", [P, plan.idx_cols], I16)
    seg_d = din("seg_all", [P, plan.nb_total], F16)
    out_d = nc.dram_tensor("out", [1, cfg.n_graphs], F32,
                           kind="ExternalOutput").ap()

    rg = [list(range(n_cores))]

    with tile.TileContext(nc) as tc:
        with tc.tile_pool(name="const", bufs=1) as cp, \
             tc.tile_pool(name="gath", bufs=2) as gp, \
             tc.tile_pool(name="band", bufs=2) as bp, \
             tc.tile_pool(name="idx", bufs=8) as ip, \
             tc.tile_pool(name="win", bufs=4) as wp, \
             tc.tile_pool(name="psS", bufs=2, space="PSUM") as ps_s, \
             tc.tile_pool(name="psT", bufs=2, space="PSUM") as ps_t, \
             tc.tile_pool(name="psX", bufs=1, space="PSUM") as ps_x, \
             tc.tile_pool(name="psP", bufs=1, space="PSUM") as ps_p, \
             tc.tile_pool(name="dram", bufs=1, space="DRAM") as dp:

            def load_const(name, ap, shape, dt=F32):
                t = cp.tile(shape, dt, tag=name)
                nc.sync.dma_start(t[:], ap[:])
                return t

            w1_s = load_const("w1", w1, [cfg.in_ch, cfg.c1])
            w2_s = load_const("w2", w2, [cfg.c1, cfg.c2], F16)
            w3_s = load_const("w3", w3, [cfg.c2, cfg.c3], F16)
            wl1_s = load_const("wl1", wl1, [cfg.c3, cfg.ch1])
            wl2_s = load_const("wl2", wl2, [cfg.ch1, 1])
            b1_s = load_const("b1", b1r, [P, cfg.c1])
            b2_s = load_const("b2", b2r, [P, cfg.c2])
            b3_s = load_const("b3", b3r, [P, cfg.c3])
            bl1_s = load_const("bl1", bl1c, [cfg.ch1, 1])
            bl2_s = load_const("bl2", bl2c, [1, 1])
            dinv_s = load_const("dinv", dinv_d, [P, NW])
            gslot_s = load_const("gslot", gslot_d, [P, NW], F16)
            invcnt_s = load_const("invcnt", invcnt_d, [cfg.c3, cfg.n_graphs])
            iota_s = load_const("iota", iota_d, [P, P], F16)
            ident_s = load_const("ident", ident_d, [P, P], F16)
            seg_s = load_const("seg", seg_d, [P, plan.nb_total], F16)
            xT_s = load_const("xT", xT, [cfg.in_ch, cfg.nsp])

            HALF = cfg.nsp // 2
            shards = {}
            fulls = {}
            for nm in ("A", "B", "C"):
                shards[nm] = (
                    dp.tile([HALF, cfg.tw], F16, name=f"ysh{nm}t"),
                    dp.tile([HALF, cfg.tw], F16, name=f"ysh{nm}b"))
                fulls[nm] = (
                    dp.tile([HALF * n_cores, cfg.tw], F16, name=f"yfl{nm}t"),
                    dp.tile([HALF * n_cores, cfg.tw], F16, name=f"yfl{nm}b"))
            HW2 = NW // 2                     # window 49: first of bottom half

            def ysh_write(tab, w, cout, y_t):
                sh = shards[tab][0 if w < HW2 else 1]
                wo = w if w < HW2 else w - HW2
                nc.sync.dma_start(sh[wo * P:(wo + 1) * P, :cout], y_t[:])
                if w == HW2 - 1:              # top half complete -> AG it
                    nc.gpsimd.collective_compute(
                        "AllGather", mybir.AluOpType.bypass,
                        replica_groups=rg,
                        ins=[shards[tab][0].opt()],
                        outs=[fulls[tab][0].opt()])

            def ag_bottom(tab):
                nc.gpsimd.collective_compute(
                    "AllGather", mybir.AluOpType.bypass, replica_groups=rg,
                    ins=[shards[tab][1].opt()], outs=[fulls[tab][1].opt()])
            pool_sh = dp.tile([cfg.c3, cfg.gslots], F32)
            pool_ag = dp.tile([cfg.c3 * n_cores, cfg.gslots], F32)

            def ts(out, in0, s1, op0, s2=None, op1=mybir.AluOpType.bypass):
                nc.vector.tensor_scalar(out, in0, s1, s2, op0, op1)

            # ---- phase 0: y1 = dinv * (x @ W1) -> table A ----
            for w in range(NW):
                ps = ps_x.tile([P, cfg.c1], F32, tag="xw")
                nc.tensor.matmul(ps[:], lhsT=xT_s[:, w * P:(w + 1) * P],
                                 rhs=w1_s[:], start=True, stop=True)
                y_t = wp.tile([P, cfg.c1], F16, tag="yt")
                ts(y_t[:], ps[:], dinv_s[:, w:w + 1], mybir.AluOpType.mult)
                ysh_write("A", w, cfg.c1, y_t)

            ag_bottom("A")

            # ---- aggregation layers ----
            # full PSUM bank: start=True clears has_written for the WHOLE
            # bank, so each logical accumulator owns a bank exclusively and
            # gets exactly one start (first matmul) / stop (last matmul).
            pool_pp = ps_p.tile([P, 512], F32, tag="pool")

            def agg_layer(lnum, cin, cout, ytab, b_s, wnext_s, ysh_next):
                for g in range(NG):
                    ws = list(range(g * GW, min((g + 1) * GW, NW)))
                    Sa = ps_s.tile([P, 512], F32, tag="Sa",
                                   name=f"Sa_{lnum}_{g}")
                    Sb = None
                    if len(ws) * cin > 512:
                        Sb = ps_s.tile([P, 512], F32, tag="Sb",
                                       name=f"Sb_{lnum}_{g}")
                    wpb = 512 // cin           # windows per bank
                    # call k's blocks live at group-tile offset koff[k]
                    koff = [plan.b0_gk[g][k] - plan.b0_gk[g][0]
                            for k in range(4)]
                    nbg = koff[3] + plan.nb_gk[g][3]

                    # first/last matmul (k, j) PER BANK for start/stop flags
                    first, last = {}, {}
                    for k in range(4):
                        for wl, w in enumerate(ws):
                            off, nb = plan.wblk[g][k][wl]
                            if nb == 0:
                                continue
                            bk = wl // wpb
                            if bk not in first:
                                first[bk] = (k, wl, off)
                            last[bk] = (k, wl, off + nb - 1)

                    gt = gp.tile([P, nbg, cfg.tw], F16, tag="gt",
                                 name=f"gt_{lnum}_{g}")
                    for k in range(4):
                        nb = plan.nb_gk[g][k]
                        if nb == 0:
                            continue
                        ni = nb * P
                        cols = ni // 16
                        it = ip.tile([P, cols], I16, tag="idxt")
                        ic0 = plan.icol0[g][k]
                        nc.sync.dma_start(it[:], idx_d[:, ic0:ic0 + cols])
                        tabh = fulls[ytab][k // 2]
                        c0r = (k % 2) * cfg.chunk
                        nc.gpsimd.dma_gather(
                            gt[:, koff[k]:koff[k] + nb, :],
                            tabh[c0r:c0r + cfg.chunk, :],
                            it[:], ni, ni, cfg.tw,
                            single_packet=False, queue_num=k)
                    band = bp.tile([P, nbg, P], F16, tag="band",
                                   name=f"band_{lnum}_{g}")
                    sg0 = plan.b0_gk[g][0]
                    nc.vector.tensor_tensor(
                        out=band[:],
                        in0=seg_s[:, sg0:sg0 + nbg].unsqueeze(2)
                            .broadcast_to([P, nbg, P]),
                        in1=iota_s[:].unsqueeze(1)
                            .broadcast_to([P, nbg, P]),
                        op=mybir.AluOpType.is_equal)
                    for k in range(4):
                        for wl, w in enumerate(ws):
                            off, nbw = plan.wblk[g][k][wl]
                            bk = wl // wpb
                            S = Sa if bk == 0 else Sb
                            col = (wl % wpb) * cin
                            for j in range(off, off + nbw):
                                nc.tensor.matmul(
                                    S[:, col:col + cin],
                                    lhsT=band[:, koff[k] + j, :],
                                    rhs=gt[:, koff[k] + j, :cin],
                                    start=(first[bk] == (k, wl, j)),
                                    stop=(last[bk] == (k, wl, j)),
                                    skip_group_check=True)

                    for wl, w in enumerate(ws):
                        bk = wl // wpb
                        S = Sa if bk == 0 else Sb
                        col = (wl % wpb) * cin
                        z = wp.tile([P, cin], F32, tag="z")
                        nc.vector.affine_then_add(
                            z[:], S[:, col:col + cin], b_s[:],
                            dinv_s[:, w:w + 1], 0.0)
                        h = wp.tile([P, cin], F16, tag="h")
                        nc.scalar.activation(
                            h[:], z[:], mybir.ActivationFunctionType.Relu)
                        if lnum < 3:
                            pst = ps_t.tile([cin, P], F16, tag="tp")
                            nc.tensor.transpose(pst[:], h[:], ident_s[:])
                            hT = wp.tile([cin, P], F16, tag="hT")
                            nc.vector.tensor_copy(hT[:], pst[:])
                            ps = ps_x.tile([P, cout], F32, tag="xw")
                            nc.tensor.matmul(ps[:], lhsT=hT[:], rhs=wnext_s[:],
                                             start=True, stop=True)
                            y_t = wp.tile([P, cout], F16, tag="yt")
                            ts(y_t[:], ps[:], dinv_s[:, w:w + 1],
                               mybir.AluOpType.mult)
                            ysh_write(ysh_next, w, cout, y_t)
                        else:
                            gb = wp.tile([P, cfg.gslots], F16, tag="gb")
                            nc.vector.tensor_tensor(
                                out=gb[:],
                                in0=gslot_s[:, w:w + 1]
                                    .broadcast_to([P, cfg.gslots]),
                                in1=iota_s[:, :cfg.gslots],
                                op=mybir.AluOpType.is_equal)
                            nc.tensor.matmul(
                                pool_pp[:cfg.c3, :cfg.gslots], lhsT=h[:],
                                rhs=gb[:], start=(w == 0), stop=(w == NW - 1),
                                skip_group_check=True)

            agg_layer(1, cfg.c1, cfg.c2, "A", b1_s, w2_s, "B")
            ag_bottom("B")
            agg_layer(2, cfg.c2, cfg.c3, "B", b2_s, w3_s, "C")
            ag_bottom("C")
            agg_layer(3, cfg.c3, None, "C", b3_s, None, None)

            # ---- pooling finale ----
            pool_t = cp.tile([cfg.c3, cfg.gslots], F32)
            nc.vector.tensor_copy(pool_t[:], pool_pp[:cfg.c3, :cfg.gslots])
            nc.sync.dma_start(pool_sh[:], pool_t[:])
            nc.gpsimd.collective_compute(
                "AllGather", mybir.AluOpType.bypass, replica_groups=rg,
                ins=[pool_sh.opt()], outs=[pool_ag.opt()])
            M = cp.tile([cfg.c3, cfg.n_graphs], F32)
            nc.vector.memset(M[:], 0.0)
            for c in range(n_cores):
                agc = wp.tile([cfg.c3, cfg.gslots], F32, tag="agc")
                nc.sync.dma_start(agc[:],
                                  pool_ag[c * cfg.c3:(c + 1) * cfg.c3, :])
                g0 = plan.g0[c]
                wdt = min(plan.gwid[c], cfg.n_graphs - g0)
                nc.vector.tensor_tensor(M[:, g0:g0 + wdt], M[:, g0:g0 + wdt],
                                        agc[:, :wdt], mybir.AluOpType.add)
            M2 = cp.tile([cfg.c3, cfg.n_graphs], F32)
            nc.vector.tensor_tensor(M2[:], M[:], invcnt_s[:],
                                    mybir.AluOpType.mult)
            ps1 = ps_t.tile([cfg.ch1, cfg.n_graphs], F32, tag="tp")
            nc.tensor.matmul(ps1[:], lhsT=wl1_s[:], rhs=M2[:],
                             start=True, stop=True)
            g1 = cp.tile([cfg.ch1, cfg.n_graphs], F32)
            ts(g1[:], ps1[:], bl1_s[:, 0:1], mybir.AluOpType.add, 0.0,
               mybir.AluOpType.max)
            ps2 = ps_x.tile([1, cfg.n_graphs], F32, tag="xw")
            nc.tensor.matmul(ps2[:], lhsT=wl2_s[:], rhs=g1[:],
                             start=True, stop=True)
            osb = cp.tile([1, cfg.n_graphs], F32)
            ts(osb[:], ps2[:], bl2_s[:, 0:1], mybir.AluOpType.add)
            nc.sync.dma_start(out_d[:], osb[:])

    nc.compile()
    return nc


def make_in_maps(cfg, plan, shared, cores, x, W1, b1, W2, b2, W3, b3,
                 Wl1, bl1, Wl2, bl2):
    NS = cfg.ns
    x = np.asarray(x, dtype=np.float32)
    com = {
        "w1": np.asarray(W1, np.float32),
        "w2b": np.asarray(W2, np.float32).astype(NPF16),
        "w3b": np.asarray(W3, np.float32).astype(NPF16),
        "wl1": np.asarray(Wl1, np.float32), "wl2": np.asarray(Wl2, np.float32),
        "b1r": np.tile(np.asarray(b1, np.float32), (P, 1)),
        "b2r": np.tile(np.asarray(b2, np.float32), (P, 1)),
        "b3r": np.tile(np.asarray(b3, np.float32), (P, 1)),
        "bl1c": np.asarray(bl1, np.float32).reshape(-1, 1),
        "bl2c": np.asarray(bl2, np.float32).reshape(1, 1),
        "invcnt_r": shared["invcnt_r"], "iota_t": shared["iota_t"],
        "ident_t": shared["ident_t"],
    }
    in_maps = []
    for c in range(cfg.n_cores):
        m = dict(com)
        xs = np.zeros((cfg.in_ch, cfg.nsp), dtype=np.float32)
        xs[:, :NS] = x[c * NS:(c + 1) * NS].T
        m["xT"] = xs
        m.update(cores[c])
        in_maps.append(m)
    return in_maps


_CACHE = {}


def _install_profile_hook():
    try:
        import antenv.axon_hooks  # noqa: F401
        return
    except ImportError:
        pass
    try:
        mod = types.ModuleType("antenv.axon_hooks")
        _h = [None]
        mod.set_axon_ntff_profile_hook = lambda h: _h.__setitem__(0, h)
        mod.get_axon_ntff_profile_hook = lambda: _h[0]
        sys.modules["antenv.axon_hooks"] = mod
        from trn_agent_boot.trn_boot import _ntff_profile_via_ctypes
        mod.set_axon_ntff_profile_hook(
            _ntff_profile_via_ctypes("/opt/axon/libaxon_pjrt.so"))
    except Exception:
        pass


def run(cfg, x, edge_index, batch, W1, b1, W2, b2, W3, b3, Wl1, bl1, Wl2, bl2,
        trace=False):
    plan, shared, cores = preprocess(cfg, edge_index, batch)
    key = ("prog", cfg.n_nodes, plan.nb_total, plan.idx_cols,
           tuple(plan.g0), tuple(plan.gwid))
    if key not in _CACHE:
        _CACHE[key] = build_program(plan, cfg.n_cores)
    nc = _CACHE[key]
    in_maps = make_in_maps(cfg, plan, shared, cores, x, W1, b1, W2, b2,
                           W3, b3, Wl1, bl1, Wl2, bl2)
    if trace:
        _install_profile_hook()
    res = run_bass_kernel_spmd(nc, in_maps, list(range(cfg.n_cores)),
                               trace=trace)
    out = np.asarray(res.results[0]["out"]).reshape(-1)[:cfg.n_graphs]
    return out.astype(np.float32), res


def kernel(x, edge_index, batch, W1, b1, W2, b2, W3, b3, Wl1, bl1, Wl2, bl2):
    cfg = Cfg()
    out, _ = run(cfg, x, edge_index, batch, W1, b1, W2, b2, W3, b3,
                 Wl1, bl1, Wl2, bl2)
    return out
